# revision 5
# baseline (speedup 1.0000x reference)
"""Trainium2 Bass kernel for nn_AttentionHead (conv3x3 -> x*am pooled -> fc6 -> fc7).

Sharding over 8 NeuronCores (single SPMD launch, 2 tiny collectives):
  - conv: data-parallel over batch (8 batches/core), 3x3 conv as matmuls with
    3 dx-shifted weight copies packed into M=96, PSUM-accumulated over 16
    CIN-chunks, DVE-combined over dy.
  - AllGather of am.T (bf16, ~800KB) so every core sees all 64 batches.
  - pooled einsum: tensor-parallel over CIN (256 ch/core); host pre-transposes
    the x chunk to [hw, cin]; 4 batches col-tiled into one PSUM bank pair.
  - fc6 with a column chunk of w6 (host pre-permuted+transposed, fp8 e3m4
    scaled x1024; the inverse scale is folded into w7), r-halves col-tiled
    onto PE column strips -> partial h6; AllReduce (fp32, 256KB); +b6, relu.
  - fc7 computed fully on every core (cheap, keeps the program uniform);
    host takes core 0's output.
Matmuls in bf16 (fc6 rhs in fp8 e3m4) with fp32 PSUM accumulation.
"""

import numpy as np
import ml_dtypes

import concourse.bass as bass
import concourse.bacc as bacc
import concourse.mybir as mybir
from concourse import tile
from concourse.bass_utils import run_bass_kernel_spmd
from concourse.masks import make_identity

F32 = mybir.dt.float32
BF16 = mybir.dt.bfloat16
FP8 = mybir.dt.float8e3
NCORES = 8
W6_SCALE = 1024.0

_NC = None


def build_module(reps=1, trace_sim=False):
    nc = bacc.Bacc(None, target_bir_lowering=False)
    xbp = nc.dram_tensor("xbp", [16, 2, 128, 4, 256], BF16, kind="ExternalInput")
    xct = nc.dram_tensor("xct", [16, 98, 4, 2, 256], BF16, kind="ExternalInput")
    cw = nc.dram_tensor("cw", [128, 16, 9, 32], BF16, kind="ExternalInput")
    cb = nc.dram_tensor("cb", [32, 1], F32, kind="ExternalInput")
    w6t = nc.dram_tensor("w6t", [16, 128, 4, 1024], FP8, kind="ExternalInput")
    b6s = nc.dram_tensor("b6s", [128, 512], F32, kind="ExternalInput")
    w7t = nc.dram_tensor("w7t", [1024, 1024], BF16, kind="ExternalInput")
    b7s = nc.dram_tensor("b7s", [128, 512], F32, kind="ExternalInput")
    outp = nc.dram_tensor("outp", [64, 1024], F32, kind="ExternalOutput")

    RG = [list(range(NCORES))]
    Relu = mybir.ActivationFunctionType.Relu
    Ident = mybir.ActivationFunctionType.Identity

    with tile.TileContext(nc, num_cores=NCORES, trace_sim=trace_sim) as tc:
        with (
            tc.tile_pool(name="consts", bufs=1) as consts,
            tc.tile_pool(name="dram", bufs=1, space="DRAM") as dram,
            tc.tile_pool(name="ps_t", bufs=2, space="PSUM") as ps_t,
            tc.tile_pool(name="ps_pool", bufs=1, space="PSUM") as ps_pool,
            tc.tile_pool(name="ps_conv", bufs=2, space="PSUM") as ps_conv,
            tc.tile_pool(name="ps_acc", bufs=2, space="PSUM") as ps_acc,
            tc.tile_pool(name="featp", bufs=2) as featp,
            tc.tile_pool(name="sb_main", bufs=1) as sb_main,
            tc.tile_pool(name="xbp_pool", bufs=18) as xbp_pool,
            tc.tile_pool(name="amio", bufs=4) as amio,
            tc.tile_pool(name="small", bufs=2) as small,
        ):
            ident = consts.tile([128, 128], BF16)
            make_identity(nc, ident[:])
            cw_sb = consts.tile([128, 16, 9, 32], BF16)
            nc.sync.dma_start(cw_sb[:], cw[:])
            cb_sb = consts.tile([32, 1], F32)
            nc.sync.dma_start(cb_sb[:], cb[:])
            # fc7 weights prefetched at t=0 so the tail never waits on them
            w7a = consts.tile([128, 4, 1024], BF16)
            nc.sync.dma_start(w7a[:], w7t[:].rearrange("(q p) r -> p q r", q=8)[:, 0:4, :])
            w7b = consts.tile([128, 4, 1024], BF16)
            nc.sync.dma_start(w7b[:], w7t[:].rearrange("(q p) r -> p q r", q=8)[:, 4:8, :])
            b6_sb = consts.tile([128, 512], F32)
            nc.sync.dma_start(b6_sb[:], b6s[:])
            b7_sb = consts.tile([128, 512], F32)
            nc.sync.dma_start(b7_sb[:], b7s[:])

            for rep in range(reps):
                # feat transposed for fc6: [kw, ih, g, (j,o)] — contiguous writes;
                # fc6 slices o via a strided lhsT AP instead
                featT = featp.tile([128, 2, 16, 128], BF16, tag="featT")

                ag_in = dram.tile([8, 196, 32], BF16)
                ag_out = dram.tile([64, 196, 32], BF16, addr_space="Shared")
                ar_in = dram.tile([64, 1024], F32)
                ar_out = dram.tile([64, 1024], F32, addr_space="Shared")

                # xct + w6 fully resident: issue all loads upfront so the DMA
                # pipe never stalls on pool write-after-read dependencies.
                with (
                    tc.tile_pool(name="xct_pool", bufs=12) as xct_pool,
                    tc.tile_pool(name="w6_pool", bufs=8) as w6_pool,
                ):
                    xv_tiles = []
                    for g in range(16):
                        t = xct_pool.tile([98, 4, 2, 256], BF16, tag="xv")
                        nc.sync.dma_start(t[:], xct[g])
                        xv_tiles.append(t)
                    w6_tiles = []
                    for kc4 in range(16):
                        t = w6_pool.tile([128, 4, 1024], FP8, tag="w6")
                        nc.sync.dma_start(t[:], w6t[kc4])
                        w6_tiles.append(t)

                    # ---------------- Phase 1: conv for own 8 batches ----------------
                    if True:
                        for pg in range(2):  # groups of 4 batches
                            xt = []
                            for kc in range(16):
                                t = xbp_pool.tile([128, 4, 256], BF16, tag="xbp")
                                nc.sync.dma_start(t[:], xbp[kc, pg])
                                xt.append(t)
                            for pj in range(2):  # batch pairs within group
                                # 144 (kc, offset) units balanced over 4 PE column
                                # strips; each strip accumulates into its 32-row
                                # PSUM slice at matching output addresses n=y*16+x.
                                ps4 = ps_conv.tile([128, 2, 224], F32, tag="cps")
                                for idx in range(144):
                                    kc, off = idx // 9, idx % 9
                                    dy, dx = off // 3, off % 3
                                    s = idx % 4
                                    w0 = dy * 16 + dx
                                    nc.tensor.matmul(
                                        ps4[32 * s:32 * s + 32, :, 0:222],
                                        cw_sb[:, kc, off, :],
                                        xt[kc][:, 2 * pj:2 * pj + 2, w0:w0 + 222],
                                        start=(idx < 4),
                                        stop=(idx >= 140),
                                        tile_position=(0, 32 * s),
                                    )
                                amacc = small.tile([32, 2, 14, 14], F32, tag="amacc")
                                S4 = ps4[:].rearrange("p a (r c) -> p a r c", r=14)
                                nc.vector.tensor_copy(amacc[:], S4[0:32, :, :, 0:14])
                                for s in range(1, 4):
                                    nc.vector.tensor_add(
                                        amacc[:], amacc[:],
                                        S4[32 * s:32 * s + 32, :, :, 0:14],
                                    )
                                ambf = amio.tile([32, 2, 14, 14], BF16, tag="ambf")
                                nc.scalar.activation(ambf[:], amacc[:], Ident, bias=cb_sb[:])
                                for b2 in range(2):
                                    amt = amio.tile([98, 2, 32], BF16, tag="amt")
                                    flat = ambf[:, b2, :, :].rearrange("p r c -> p (r c)")
                                    for h in range(2):
                                        tp = ps_t.tile([98, 32], BF16, tag="tp")
                                        nc.tensor.transpose(
                                            tp[:], flat[:, h * 98:(h + 1) * 98], ident[:32, :32]
                                        )
                                        nc.vector.tensor_copy(amt[:, h, :], tp[:])
                                    b = 4 * pg + 2 * pj + b2
                                    nc.sync.dma_start(
                                        ag_in[b, :, :].rearrange("(h p) o -> p h o", h=2), amt[:]
                                    )

                    # ---------------- Phase 2: AllGather am.T ----------------
                    nc.gpsimd.collective_compute(
                        "AllGather", mybir.AluOpType.bypass,
                        replica_groups=RG, ins=[ag_in[:]], outs=[ag_out[:]],
                    )
                    # all 64 batches' am.T in SBUF: [p, h, b, o]
                    amT = sb_main.tile([98, 2, 64, 32], BF16)
                    for h in range(2):
                        nc.sync.dma_start(
                            amT[:, h, :, :],
                            ag_out[:, h * 98:(h + 1) * 98, :].rearrange("b p o -> p b o"),
                        )

                    # ---- Phase 3: pooled einsum, 4 batches col-tiled per PSUM bank ----
                    for g in range(16):
                        xv = xv_tiles[g]
                        psA = ps_pool.tile([128, 256], F32, tag="ppsA")
                        psB = ps_pool.tile([128, 256], F32, tag="ppsB")
                        for j in range(4):
                            b = 4 * g + j
                            nc.tensor.matmul(
                                psA[32 * j:32 * j + 32, :], amT[:, 0, b, :], xv[:, j, 0, :],
                                start=True, stop=True, tile_position=(0, 32 * j),
                            )
                            nc.tensor.matmul(
                                psB[32 * j:32 * j + 32, :], amT[:, 1, b, :], xv[:, j, 1, :],
                                start=True, stop=True, tile_position=(0, 32 * j),
                            )
                        stage = small.tile([128, 256], F32, tag="po4")
                        nc.scalar.activation(stage[:], psA[:], Ident)
                        stage2 = small.tile([128, 256], BF16, tag="po4b")
                        nc.vector.tensor_add(stage2[:], stage[:], psB[:])
                        # transpose [(j,o), i-half] -> [i-half, (j,o)] and scatter
                        # into featT[:, kc=(ih*32+o), b=4g+j]
                        for ih in range(2):
                            tp = ps_t.tile([128, 128], BF16, tag="tp")
                            nc.tensor.transpose(
                                tp[:], stage2[:, ih * 128:(ih + 1) * 128], ident[:]
                            )
                            nc.vector.tensor_copy(featT[:, ih, g, :], tp[:])

                    # ---- Phase 5: fc6 (col-tiled r-halves) + AllReduce + relu ----
                    if True:
                        h6A = ps_acc.tile([128, 512], F32, tag="acc", name="h6A")
                        h6B = ps_acc.tile([128, 512], F32, tag="acc", name="h6B")
                        for kc4 in range(16):
                            wt = w6_tiles[kc4]
                            for q in range(4):
                                kc = 4 * kc4 + q
                                ih, o = kc // 32, kc % 32
                                lhs = featT[:].rearrange(
                                    "p h g (j o) -> p h g o j", j=4
                                )[:, ih, :, o, :]
                                nc.tensor.matmul(
                                    h6A[0:64, :], lhs, wt[:, q, 0:512],
                                    start=(kc == 0), stop=(kc == 63), tile_position=(0, 0),
                                )
                                nc.tensor.matmul(
                                    h6B[64:128, :], lhs, wt[:, q, 512:1024],
                                    start=(kc == 0), stop=(kc == 63), tile_position=(0, 64),
                                )
                        # h6sb rows 0-63: r 0-511 (h6A), rows 64-127: r 512-1023 (h6B)
                        h6sb = sb_main.tile([128, 512], F32)
                        nc.vector.tensor_add(h6sb[0:64, :], h6A[0:64, :], b6_sb[0:64, :])
                        nc.vector.tensor_add(h6sb[64:128, :], h6B[64:128, :], b6_sb[64:128, :])
                        nc.sync.dma_start(
                            ar_in[:].rearrange("b (s r) -> s b r", s=2), h6sb[:]
                        )
                        nc.gpsimd.collective_compute(
                            "AllReduce", mybir.AluOpType.add,
                            replica_groups=RG, ins=[ar_in[:]], outs=[ar_out[:]],
                        )
                        h6r = sb_main.tile([64, 1024], F32)
                        nc.sync.dma_start(h6r[:], ar_out[:])
                        h6a = sb_main.tile([64, 1024], BF16)
                        nc.scalar.activation(h6a[:], h6r[:], Relu)

                        # ------------- Phase 6: fc7 on all 64 batches -------------
                        h7T = sb_main.tile([128, 8, 64], BF16)
                        for k7 in range(8):
                            tp = ps_t.tile([128, 64], BF16, tag="tp")
                            nc.tensor.transpose(
                                tp[:], h6a[:, k7 * 128:(k7 + 1) * 128], ident[:64, :64]
                            )
                            nc.vector.tensor_copy(h7T[:, k7, :], tp[:])
                        opsA = ps_acc.tile([128, 512], F32, tag="acc", name="opsA")
                        opsB = ps_acc.tile([128, 512], F32, tag="acc", name="opsB")
                        for k7 in range(8):
                            wsrc = w7a if k7 < 4 else w7b
                            nc.tensor.matmul(
                                opsA[0:64, :], h7T[:, k7, :], wsrc[:, k7 % 4, 0:512],
                                start=(k7 == 0), stop=(k7 == 7), tile_position=(0, 0),
                            )
                            nc.tensor.matmul(
                                opsB[64:128, :], h7T[:, k7, :], wsrc[:, k7 % 4, 512:1024],
                                start=(k7 == 0), stop=(k7 == 7), tile_position=(0, 64),
                            )
                        t12 = small.tile([128, 512], F32, tag="t12")
                        nc.vector.tensor_add(t12[0:64, :], opsA[0:64, :], b7_sb[0:64, :])
                        nc.vector.tensor_add(t12[64:128, :], opsB[64:128, :], b7_sb[64:128, :])
                        osb = sb_main.tile([128, 512], F32)
                        nc.scalar.activation(osb[:], t12[:], Relu)
                        nc.sync.dma_start(
                            outp[:].rearrange("b (s r) -> s b r", s=2), osb[:]
                        )

    nc.compile()
    return nc


def _bf(a):
    return np.ascontiguousarray(a).astype(ml_dtypes.bfloat16)


def _f32(a):
    return np.ascontiguousarray(a).astype(np.float32)


def _fp8(a):
    return np.ascontiguousarray(np.clip(a, -15.5, 15.5)).astype(ml_dtypes.float8_e3m4)


def prep_inputs(x, conv_w, conv_b, w6, b6, w7, b7):
    x = np.asarray(x, np.float32)
    conv_w = np.asarray(conv_w, np.float32)
    conv_b = np.asarray(conv_b, np.float32)
    w6 = np.asarray(w6, np.float32)
    b6 = np.asarray(b6, np.float32)
    w7 = np.asarray(w7, np.float32)
    b7 = np.asarray(b7, np.float32)

    xp = np.zeros((64, 2048, 16, 16), np.float32)
    xp[:, :, 1:15, 1:15] = x
    xpg = xp.reshape(64, 2048, 256)
    w6r = (w6 * W6_SCALE).reshape(1024, 2048, 32)
    cwt = (conv_w / 196.0).reshape(32, 16, 128, 3, 3).transpose(2, 1, 3, 4, 0)
    cw = np.ascontiguousarray(cwt).reshape(128, 16, 9, 32)
    cb = (conv_b / 196.0).reshape(32, 1)
    # stacked bias layouts: rows 0-63 get [0:512], rows 64-127 get [512:1024]
    # b6 carries the w6 fp8 pre-scale; w7 carries the inverse.
    b6s_ = np.empty((128, 512), np.float32)
    b6s_[0:64, :] = (b6[0:512] * (W6_SCALE / NCORES))[None, :]
    b6s_[64:128, :] = (b6[512:1024] * (W6_SCALE / NCORES))[None, :]
    b7s_ = np.empty((128, 512), np.float32)
    b7s_[0:64, :] = b7[0:512][None, :]
    b7s_[64:128, :] = b7[512:1024][None, :]
    w7t = w7.T / W6_SCALE

    cw_b = _bf(cw)
    cb_f = _f32(cb)
    b6_f = _f32(b6s_)
    b7_f = _f32(b7s_)
    w7_b = _bf(w7t)

    in_maps = []
    for c in range(NCORES):
        bs = slice(8 * c, 8 * c + 8)
        i0 = 256 * c
        arr8 = xpg[bs].reshape(8, 16, 128, 256)
        xbp_ = arr8.transpose(1, 0, 2, 3).reshape(16, 2, 4, 128, 256).transpose(0, 1, 3, 2, 4)
        xs = x[:, i0:i0 + 256].reshape(64, 256, 196)
        xct_o = xs.transpose(0, 2, 1).reshape(64, 2, 98, 256).transpose(0, 2, 1, 3)
        xct_ = xct_o.reshape(16, 4, 98, 2, 256).transpose(0, 2, 1, 3, 4)
        w6c = w6r[:, i0:i0 + 256, :]  # [r, il, o]
        # row order: kc = (il//128)*32 + o, p = il%128
        w6t_o = np.ascontiguousarray(
            w6c.reshape(1024, 2, 128, 32).transpose(1, 3, 2, 0)
        ).reshape(8192, 1024)
        w6t_ = w6t_o.reshape(16, 4, 128, 1024).transpose(0, 2, 1, 3)
        in_maps.append(dict(
            xbp=_bf(xbp_), xct=_bf(xct_), cw=cw_b, cb=cb_f,
            w6t=_fp8(w6t_), b6s=b6_f, w7t=w7_b, b7s=b7_f,
        ))
    return in_maps


def run(in_maps, **kwargs):
    global _NC
    if _NC is None:
        _NC = build_module()
    return run_bass_kernel_spmd(_NC, in_maps, list(range(NCORES)), **kwargs)


def kernel(x, conv_w, conv_b, w6, b6, w7, b7):
    in_maps = prep_inputs(x, conv_w, conv_b, w6, b6, w7, b7)
    res = run(in_maps)
    return np.asarray(res.results[0]["outp"], dtype=np.float32)


# revision 6
# speedup vs baseline: 1.9122x; 1.9122x over previous
"""Trainium2 Bass kernel for nn_AttentionHead (conv3x3 -> x*am pooled -> fc6 -> fc7).

Sharding over 8 NeuronCores (single SPMD launch, 2 tiny collectives):
  - conv: data-parallel over batch (8 batches/core), 3x3 conv as matmuls with
    shifted-window PSUM accumulation over 16 CIN-chunks and 9 taps, 4-way
    column-strip concurrency.
  - AllGather of am.T (bf16, ~100KB/core) so every core sees all 64 batches.
  - pooled einsum: tensor-parallel over CIN (256 ch/core); host pre-transposes
    the x chunk to [hw, cin]; 4 batches col-tiled, both hw-halves PSUM-summed.
  - fc6 with a column chunk of w6 (host pre-permuted+transposed, fp8 e3m4
    scaled x1024; the inverse scale is folded into w7), r-halves col-tiled
    onto PE column strips -> partial h6; AllReduce (fp32, 256KB); +b6, relu.
  - fc7 computed fully on every core (cheap, keeps the program uniform).
The rep loop is software-pipelined: rep k's AllReduce/fc7 tail is issued
AFTER rep k+1's conv+AllGather, so the two collectives of consecutive reps
interleave on the collective channel (AG_{k+1} precedes AR_k) and the tail
overlaps the next rep's head.
"""

import numpy as np
import ml_dtypes

import concourse.bass as bass
import concourse.bacc as bacc
import concourse.mybir as mybir
from concourse import tile
from concourse.bass_utils import run_bass_kernel_spmd
from concourse.masks import make_identity

F32 = mybir.dt.float32
BF16 = mybir.dt.bfloat16
FP8 = mybir.dt.float8e3
NCORES = 8
W6_SCALE = 1024.0

_NC = None


def build_module(reps=1, trace_sim=False):
    nc = bacc.Bacc(None, target_bir_lowering=False)
    xbp = nc.dram_tensor("xbp", [16, 2, 128, 4, 256], BF16, kind="ExternalInput")
    xct = nc.dram_tensor("xct", [16, 98, 4, 2, 256], BF16, kind="ExternalInput")
    cw = nc.dram_tensor("cw", [128, 16, 9, 32], BF16, kind="ExternalInput")
    cb = nc.dram_tensor("cb", [32, 1], F32, kind="ExternalInput")
    w6t = nc.dram_tensor("w6t", [16, 128, 4, 1024], FP8, kind="ExternalInput")
    b6s = nc.dram_tensor("b6s", [128, 512], F32, kind="ExternalInput")
    w7t = nc.dram_tensor("w7t", [1024, 1024], BF16, kind="ExternalInput")
    b7s = nc.dram_tensor("b7s", [128, 512], F32, kind="ExternalInput")
    outp = nc.dram_tensor("outp", [64, 1024], F32, kind="ExternalOutput")

    RG = [list(range(NCORES))]
    Relu = mybir.ActivationFunctionType.Relu
    Ident = mybir.ActivationFunctionType.Identity

    with tile.TileContext(nc, num_cores=NCORES, trace_sim=trace_sim) as tc:
        with (
            tc.tile_pool(name="consts", bufs=1) as consts,
            tc.tile_pool(name="dram", bufs=2, space="DRAM") as dram,
            tc.tile_pool(name="ps_t", bufs=2, space="PSUM") as ps_t,
            tc.tile_pool(name="ps_pool", bufs=2, space="PSUM") as ps_pool,
            tc.tile_pool(name="ps_conv", bufs=2, space="PSUM") as ps_conv,
            tc.tile_pool(name="ps_acc", bufs=2, space="PSUM") as ps_acc,
            tc.tile_pool(name="featp", bufs=2) as featp,
            tc.tile_pool(name="sb_main", bufs=1) as sb_main,
            tc.tile_pool(name="xbp_pool", bufs=18) as xbp_pool,
            tc.tile_pool(name="xct_pool", bufs=12) as xct_pool,
            tc.tile_pool(name="w6_pool", bufs=8) as w6_pool,
            tc.tile_pool(name="amio", bufs=2) as amio,
            tc.tile_pool(name="small", bufs=2) as small,
        ):
            ident = consts.tile([128, 128], BF16)
            make_identity(nc, ident[:])
            cw_sb = consts.tile([128, 16, 9, 32], BF16)
            nc.sync.dma_start(cw_sb[:], cw[:])
            cb_sb = consts.tile([32, 1], F32)
            nc.sync.dma_start(cb_sb[:], cb[:])
            # fc7 weights prefetched at t=0 so the tail never waits on them
            w7a = consts.tile([128, 4, 1024], BF16)
            nc.sync.dma_start(w7a[:], w7t[:].rearrange("(q p) r -> p q r", q=8)[:, 0:4, :])
            w7b = consts.tile([128, 4, 1024], BF16)
            nc.sync.dma_start(w7b[:], w7t[:].rearrange("(q p) r -> p q r", q=8)[:, 4:8, :])
            b6_sb = consts.tile([128, 512], F32)
            nc.sync.dma_start(b6_sb[:], b6s[:])
            b7_sb = consts.tile([128, 512], F32)
            nc.sync.dma_start(b7_sb[:], b7s[:])

            def tail_phase(ar_in, ar_out):
                # AllReduce partial h6, then relu -> fc7 -> out
                nc.gpsimd.collective_compute(
                    "AllReduce", mybir.AluOpType.add,
                    replica_groups=RG, ins=[ar_in[:]], outs=[ar_out[:]],
                )
                h6r = sb_main.tile([64, 1024], F32, tag="h6r")
                nc.sync.dma_start(h6r[:], ar_out[:])
                h6a = sb_main.tile([64, 1024], BF16, tag="h6a")
                nc.scalar.activation(h6a[:], h6r[:], Relu)
                h7T = sb_main.tile([128, 8, 64], BF16, tag="h7T")
                for k7 in range(8):
                    tp = ps_t.tile([128, 64], BF16, tag="tp")
                    nc.tensor.transpose(
                        tp[:], h6a[:, k7 * 128:(k7 + 1) * 128], ident[:64, :64]
                    )
                    nc.vector.tensor_copy(h7T[:, k7, :], tp[:])
                opsA = ps_acc.tile([128, 512], F32, tag="acc", name="opsA")
                opsB = ps_acc.tile([128, 512], F32, tag="acc", name="opsB")
                for k7 in range(8):
                    wsrc = w7a if k7 < 4 else w7b
                    nc.tensor.matmul(
                        opsA[0:64, :], h7T[:, k7, :], wsrc[:, k7 % 4, 0:512],
                        start=(k7 == 0), stop=(k7 == 7), tile_position=(0, 0),
                    )
                    nc.tensor.matmul(
                        opsB[64:128, :], h7T[:, k7, :], wsrc[:, k7 % 4, 512:1024],
                        start=(k7 == 0), stop=(k7 == 7), tile_position=(0, 64),
                    )
                t12 = small.tile([128, 512], F32, tag="t12")
                nc.vector.tensor_add(t12[0:64, :], opsA[0:64, :], b7_sb[0:64, :])
                nc.vector.tensor_add(t12[64:128, :], opsB[64:128, :], b7_sb[64:128, :])
                osb = sb_main.tile([128, 512], F32, tag="osb")
                nc.scalar.activation(osb[:], t12[:], Relu)
                nc.sync.dma_start(
                    outp[:].rearrange("b (s r) -> s b r", s=2), osb[:]
                )

            pending = None
            for rep in range(reps):
                featT = featp.tile([128, 2, 16, 128], BF16, tag="featT")
                ag_in = dram.tile([8, 196, 32], BF16, tag="ag_in")
                ag_out = dram.tile([64, 196, 32], BF16, addr_space="Shared", tag="ag_out")
                ar_in = dram.tile([64, 1024], F32, tag="ar_in")
                ar_out = dram.tile([64, 1024], F32, addr_space="Shared", tag="ar_out")

                # xct + w6 streamed through rings; issue all loads upfront
                xv_tiles = []
                for g in range(16):
                    t = xct_pool.tile([98, 4, 2, 256], BF16, tag="xv")
                    nc.sync.dma_start(t[:], xct[g])
                    xv_tiles.append(t)
                w6_tiles = []
                for kc4 in range(16):
                    t = w6_pool.tile([128, 4, 1024], FP8, tag="w6")
                    nc.sync.dma_start(t[:], w6t[kc4])
                    w6_tiles.append(t)

                # ---------------- Phase 1: conv for own 8 batches ----------------
                for pg in range(2):  # groups of 4 batches
                    xt = []
                    for kc in range(16):
                        t = xbp_pool.tile([128, 4, 256], BF16, tag="xbp")
                        nc.sync.dma_start(t[:], xbp[kc, pg])
                        xt.append(t)
                    amt4 = amio.tile([98, 4, 2, 32], BF16, tag="amt4")
                    for pj in range(2):  # batch pairs within group
                        # 144 (kc, offset) units balanced over 4 PE column
                        # strips; each strip accumulates into its 32-row
                        # PSUM slice at matching output addresses n=y*16+x.
                        ps4 = ps_conv.tile([128, 2, 224], F32, tag="cps")
                        for idx in range(144):
                            kc, off = idx // 9, idx % 9
                            dy, dx = off // 3, off % 3
                            s = idx % 4
                            w0 = dy * 16 + dx
                            nc.tensor.matmul(
                                ps4[32 * s:32 * s + 32, :, 0:222],
                                cw_sb[:, kc, off, :],
                                xt[kc][:, 2 * pj:2 * pj + 2, w0:w0 + 222],
                                start=(idx < 4),
                                stop=(idx >= 140),
                                tile_position=(0, 32 * s),
                            )
                        amacc = small.tile([32, 2, 14, 14], F32, tag="amacc")
                        S4 = ps4[:].rearrange("p a (r c) -> p a r c", r=14)
                        nc.vector.tensor_copy(amacc[:], S4[0:32, :, :, 0:14])
                        for s in range(1, 4):
                            nc.vector.tensor_add(
                                amacc[:], amacc[:],
                                S4[32 * s:32 * s + 32, :, :, 0:14],
                            )
                        ambf = amio.tile([32, 2, 14, 14], BF16, tag="ambf")
                        nc.scalar.activation(ambf[:], amacc[:], Ident, bias=cb_sb[:])
                        for b2 in range(2):
                            flat = ambf[:, b2, :, :].rearrange("p r c -> p (r c)")
                            for h in range(2):
                                tp = ps_t.tile([98, 32], BF16, tag="tp")
                                nc.tensor.transpose(
                                    tp[:], flat[:, h * 98:(h + 1) * 98], ident[:32, :32]
                                )
                                nc.vector.tensor_copy(amt4[:, 2 * pj + b2, h, :], tp[:])
                    nc.sync.dma_start(
                        ag_in[4 * pg:4 * pg + 4].rearrange("b (h p) o -> p b h o", h=2),
                        amt4[:],
                    )

                # ---------------- Phase 2: AllGather am.T ----------------
                nc.gpsimd.collective_compute(
                    "AllGather", mybir.AluOpType.bypass,
                    replica_groups=RG, ins=[ag_in[:]], outs=[ag_out[:]],
                )

                # Software pipeline: previous rep's AllReduce + fc7 tail goes
                # here, AFTER this rep's AllGather is on the channel.
                if pending is not None:
                    tail_phase(*pending)
                    pending = None

                # all 64 batches' am.T in SBUF: [p, h, b, o]
                amT = sb_main.tile([98, 2, 64, 32], BF16, tag="amT")
                for h in range(2):
                    nc.sync.dma_start(
                        amT[:, h, :, :],
                        ag_out[:, h * 98:(h + 1) * 98, :].rearrange("b p o -> p b o"),
                    )

                # ---- Phase 3: pooled einsum, 4 batches col-tiled per bank ----
                for g in range(16):
                    xv = xv_tiles[g]
                    ps = ps_pool.tile([128, 256], F32, tag="pps")
                    for j in range(4):
                        b = 4 * g + j
                        nc.tensor.matmul(
                            ps[32 * j:32 * j + 32, :], amT[:, 0, b, :], xv[:, j, 0, :],
                            start=True, stop=False, tile_position=(0, 32 * j),
                        )
                        nc.tensor.matmul(
                            ps[32 * j:32 * j + 32, :], amT[:, 1, b, :], xv[:, j, 1, :],
                            start=False, stop=True, tile_position=(0, 32 * j),
                        )
                    stage2 = small.tile([128, 256], BF16, tag="po4b")
                    nc.scalar.activation(stage2[:], ps[:], Ident)
                    # transpose [(j,o), i-half] -> [i-half, (j,o)] and scatter
                    # into featT[:, kc=(ih*32+o), b=4g+j]
                    for ih in range(2):
                        tp = ps_t.tile([128, 128], BF16, tag="tp")
                        nc.tensor.transpose(
                            tp[:], stage2[:, ih * 128:(ih + 1) * 128], ident[:]
                        )
                        nc.vector.tensor_copy(featT[:, ih, g, :], tp[:])

                # ---- Phase 5: fc6 (col-tiled r-halves) -> partial h6 ----
                h6A = ps_acc.tile([128, 512], F32, tag="acc", name="h6A")
                h6B = ps_acc.tile([128, 512], F32, tag="acc", name="h6B")
                for kc4 in range(16):
                    wt = w6_tiles[kc4]
                    for q in range(4):
                        kc = 4 * kc4 + q
                        ih, o = kc // 32, kc % 32
                        lhs = featT[:].rearrange(
                            "p h g (j o) -> p h g o j", j=4
                        )[:, ih, :, o, :]
                        nc.tensor.matmul(
                            h6A[0:64, :], lhs, wt[:, q, 0:512],
                            start=(kc == 0), stop=(kc == 63), tile_position=(0, 0),
                        )
                        nc.tensor.matmul(
                            h6B[64:128, :], lhs, wt[:, q, 512:1024],
                            start=(kc == 0), stop=(kc == 63), tile_position=(0, 64),
                        )
                # h6sb rows 0-63: r 0-511 (h6A), rows 64-127: r 512-1023 (h6B)
                h6sb = sb_main.tile([128, 512], F32, tag="h6sb")
                nc.vector.tensor_add(h6sb[0:64, :], h6A[0:64, :], b6_sb[0:64, :])
                nc.vector.tensor_add(h6sb[64:128, :], h6B[64:128, :], b6_sb[64:128, :])
                nc.sync.dma_start(
                    ar_in[:].rearrange("b (s r) -> s b r", s=2), h6sb[:]
                )
                pending = (ar_in, ar_out)

            tail_phase(*pending)

    nc.compile()
    return nc


def _bf(a):
    return np.ascontiguousarray(a).astype(ml_dtypes.bfloat16)


def _f32(a):
    return np.ascontiguousarray(a).astype(np.float32)


def _fp8(a):
    return np.ascontiguousarray(np.clip(a, -15.5, 15.5)).astype(ml_dtypes.float8_e3m4)


def prep_inputs(x, conv_w, conv_b, w6, b6, w7, b7):
    x = np.asarray(x, np.float32)
    conv_w = np.asarray(conv_w, np.float32)
    conv_b = np.asarray(conv_b, np.float32)
    w6 = np.asarray(w6, np.float32)
    b6 = np.asarray(b6, np.float32)
    w7 = np.asarray(w7, np.float32)
    b7 = np.asarray(b7, np.float32)

    xp = np.zeros((64, 2048, 16, 16), np.float32)
    xp[:, :, 1:15, 1:15] = x
    xpg = xp.reshape(64, 2048, 256)
    w6r = (w6 * W6_SCALE).reshape(1024, 2048, 32)
    cwt = (conv_w / 196.0).reshape(32, 16, 128, 3, 3).transpose(2, 1, 3, 4, 0)
    cw = np.ascontiguousarray(cwt).reshape(128, 16, 9, 32)
    cb = (conv_b / 196.0).reshape(32, 1)
    # stacked bias layouts: rows 0-63 get [0:512], rows 64-127 get [512:1024]
    # b6 carries the w6 fp8 pre-scale; w7 carries the inverse.
    b6s_ = np.empty((128, 512), np.float32)
    b6s_[0:64, :] = (b6[0:512] * (W6_SCALE / NCORES))[None, :]
    b6s_[64:128, :] = (b6[512:1024] * (W6_SCALE / NCORES))[None, :]
    b7s_ = np.empty((128, 512), np.float32)
    b7s_[0:64, :] = b7[0:512][None, :]
    b7s_[64:128, :] = b7[512:1024][None, :]
    w7t = w7.T / W6_SCALE

    cw_b = _bf(cw)
    cb_f = _f32(cb)
    b6_f = _f32(b6s_)
    b7_f = _f32(b7s_)
    w7_b = _bf(w7t)

    in_maps = []
    for c in range(NCORES):
        bs = slice(8 * c, 8 * c + 8)
        i0 = 256 * c
        arr8 = xpg[bs].reshape(8, 16, 128, 256)
        xbp_ = arr8.transpose(1, 0, 2, 3).reshape(16, 2, 4, 128, 256).transpose(0, 1, 3, 2, 4)
        xs = x[:, i0:i0 + 256].reshape(64, 256, 196)
        xct_o = xs.transpose(0, 2, 1).reshape(64, 2, 98, 256).transpose(0, 2, 1, 3)
        xct_ = xct_o.reshape(16, 4, 98, 2, 256).transpose(0, 2, 1, 3, 4)
        w6c = w6r[:, i0:i0 + 256, :]  # [r, il, o]
        # row order: kc = (il//128)*32 + o, p = il%128
        w6t_o = np.ascontiguousarray(
            w6c.reshape(1024, 2, 128, 32).transpose(1, 3, 2, 0)
        ).reshape(8192, 1024)
        w6t_ = w6t_o.reshape(16, 4, 128, 1024).transpose(0, 2, 1, 3)
        in_maps.append(dict(
            xbp=_bf(xbp_), xct=_bf(xct_), cw=cw_b, cb=cb_f,
            w6t=_fp8(w6t_), b6s=b6_f, w7t=w7_b, b7s=b7_f,
        ))
    return in_maps


def run(in_maps, **kwargs):
    global _NC
    if _NC is None:
        _NC = build_module()
    return run_bass_kernel_spmd(_NC, in_maps, list(range(NCORES)), **kwargs)


def kernel(x, conv_w, conv_b, w6, b6, w7, b7):
    in_maps = prep_inputs(x, conv_w, conv_b, w6, b6, w7, b7)
    res = run(in_maps)
    return np.asarray(res.results[0]["outp"], dtype=np.float32)
